# revision 13
# baseline (speedup 1.0000x reference)
"""Randomized Hadamard transform kernel for Trainium2 (8 NeuronCores, SPMD).

Math: out = FWHT(x * seed) / sqrt(4096); the reference butterfly equals the
Sylvester Hadamard matrix, and H_4096 = H_32 (x) H_128 (c = hi*128 + lo):

    out[r, j*128 + l] = (1/64) * sum_{hi,lo} H32[hi,j] H128[lo,l] x~[r, hi*128+lo]

Layout trick: matmul computes out[m, n] = sum_k lhsT[k, m] * rhs[k, n].
With the *data* as stationary lhsT and the Hadamard factor as moving rhs,
one MM both contracts the data's partition index and rotates a 128-wide
free window onto the output partitions. Two such passes apply both factors
and land in a store-friendly orientation — no explicit transposes.

Per 128-row tile (rows r = r0 + r_hi*4 + r_lo, cols c = hi*128 + lo):
  load   Lt[(r_lo,hi), (r_hi,lo)] <- x   in 4 quarter-DMAs (32-row slabs)
  mult   Xt = Lt * S_rep  per quarter (DVE; S_rep is r_hi-periodic -> [128,1024])
  pass1  chunk r_hi: psum[lo, (r_lo,j)] = sum_(r_lo,hi) Xt[(r_lo,hi), lo] * (I4 (x) H32)
  pass2  chunk r_hi: psum[(r_lo,j), l] = sum_lo W[lo, (r_lo,j)] * (H128/64)
  store  O[(r_lo,j), (r_hi,l)] -> out  in 2 half-DMAs (64-row slabs)
All DMA chunks are 512B-contiguous.
"""

import os

import ml_dtypes
import numpy as np

import concourse.mybir as mybir
from concourse import bacc
import concourse.tile as tile
from concourse.bass_utils import run_bass_kernel_spmd

N_CORES = 8
R_FULL = 8192
C = 4096
R_CORE = R_FULL // N_CORES  # 1024 rows per core
P = 128
NHI, NLO, NRL, NRH = 32, 128, 4, 32  # c = hi*128+lo ; tile rows = r_hi*4+r_lo
QF = 1024  # quarter free-size (8 r_hi chunks)


def _sylvester(n: int) -> np.ndarray:
    h = np.array([[1.0]], dtype=np.float64)
    while h.shape[0] < n:
        h = np.block([[h, h], [h, -h]])
    return h


def _consts():
    k1 = np.kron(np.eye(NRL), _sylvester(NHI)).astype(ml_dtypes.bfloat16)
    k2 = (_sylvester(NLO) / 64.0).astype(ml_dtypes.bfloat16)
    return k1, k2


def build_nc(
    rows: int = R_CORE,
    *,
    lt_bufs: int = 8,
    o_bufs: int = 4,
    gps_mul: bool = True,
    quarter_stores: bool = False,
):
    assert rows % P == 0
    n_tiles = rows // P

    k1_np, k2_np = _consts()

    nc = bacc.Bacc("TRN2", target_bir_lowering=False, debug=False)
    x_in = nc.dram_tensor("x", [rows, C], mybir.dt.float32, kind="ExternalInput")
    s_in = nc.dram_tensor("seed", [NHI, NLO], mybir.dt.float32, kind="ExternalInput")
    y_out = nc.dram_tensor("y", [rows, C], mybir.dt.float32, kind="ExternalOutput")
    k1_dram = nc.inline_tensor(k1_np, "k1")
    k2_dram = nc.inline_tensor(k2_np, "k2")

    f32 = mybir.dt.float32
    bf16 = mybir.dt.bfloat16

    with tile.TileContext(nc) as tc:
        with (
            tc.tile_pool(name="consts", bufs=1) as cpool,
            tc.tile_pool(name="lt", bufs=lt_bufs) as lt_pool,
            tc.tile_pool(name="xt", bufs=lt_bufs) as xt_pool,
            tc.tile_pool(name="w", bufs=3) as w_pool,
            tc.tile_pool(name="o", bufs=o_bufs) as o_pool,
            tc.tile_pool(name="ps1", bufs=4, space="PSUM") as ps1_pool,
            tc.tile_pool(name="ps2", bufs=4, space="PSUM") as ps2_pool,
        ):
            k1 = cpool.tile([P, P], bf16)
            k2 = cpool.tile([P, P], bf16)
            srep = cpool.tile([P, QF], f32)
            # constants ride the Scalar HWDGE ring so the first x load
            # starts immediately on the Sync ring
            nc.scalar.dma_start(out=k1[:], in_=k1_dram[:])
            nc.scalar.dma_start(out=k2[:], in_=k2_dram[:])
            nc.scalar.dma_start(out=srep[:NHI, :NLO], in_=s_in[:])
            # replicate seed tile on-chip: 4x along partitions, 8x along free
            for r in range(1, NRL):
                nc.vector.tensor_copy(
                    out=srep[r * NHI : (r + 1) * NHI, :NLO], in_=srep[:NHI, :NLO]
                )
            for d in range(3):
                w0 = NLO << d
                nc.vector.tensor_copy(out=srep[:, w0 : 2 * w0], in_=srep[:, :w0])

            for t in range(n_tiles):
                r0 = t * P
                # ---- load + seed multiply (fp32 -> bf16), in 4 quarters
                xtq = []
                for qi in range(4):
                    ltq = lt_pool.tile([P, QF], f32, tag="ltq")
                    src = x_in[r0 + 32 * qi : r0 + 32 * (qi + 1), :].rearrange(
                        "(rh rl) (hi lo) -> rl hi rh lo", rl=NRL, lo=NLO
                    )
                    nc.sync.dma_start(out=ltq[:], in_=src)
                    xq = xt_pool.tile([P, QF], bf16, tag="xtq")
                    # alternate the seed-multiply between DVE and GpSimd
                    eng = nc.vector if (qi % 2 == 0 or not gps_mul) else nc.gpsimd
                    eng.tensor_mul(out=xq[:], in0=ltq[:], in1=srep[:])
                    xtq.append(xq)

                # ---- pass 1: contract (r_lo,hi) with I4 (x) H32; lo -> partitions
                w = w_pool.tile([P, C], bf16)
                for g in range(NRH // 4):
                    ps = ps1_pool.tile([P, 512], f32)
                    for q in range(4):
                        rh = 4 * g + q
                        nc.tensor.matmul(
                            ps[:, q * P : (q + 1) * P],
                            lhsT=xtq[rh // 8][:, (rh % 8) * P : (rh % 8 + 1) * P],
                            rhs=k1[:],
                            start=True,
                            stop=True,
                        )
                    if g % 2 == 0:
                        nc.vector.tensor_copy(out=w[:, g * 512 : (g + 1) * 512], in_=ps[:])
                    else:
                        nc.scalar.copy(out=w[:, g * 512 : (g + 1) * 512], in_=ps[:])

                # ---- pass 2: contract lo with H128/64; (r_lo,j) -> partitions
                gper = 2 if quarter_stores else 4  # psum groups per store
                for g in range(NRH // 4):
                    if g % gper == 0:
                        oq = o_pool.tile(
                            [P, 512 * gper], f32, tag="oq", name=f"oq{t}_{g // gper}"
                        )
                    ps = ps2_pool.tile([P, 512], f32)
                    for q in range(4):
                        rh = 4 * g + q
                        nc.tensor.matmul(
                            ps[:, q * P : (q + 1) * P],
                            lhsT=w[:, rh * P : (rh + 1) * P],
                            rhs=k2[:],
                            start=True,
                            stop=True,
                        )
                    dst_sb = oq[:, (g % gper) * 512 : (g % gper + 1) * 512]
                    if g % 2 == 1:
                        nc.vector.tensor_copy(out=dst_sb, in_=ps[:])
                    else:
                        nc.scalar.copy(out=dst_sb, in_=ps[:])
                    # ---- store each slab as soon as it is drained
                    if g % gper == gper - 1:
                        h = g // gper
                        nrow = 16 * gper
                        dst = y_out[r0 + nrow * h : r0 + nrow * (h + 1), :].rearrange(
                            "(rh rl) (j l) -> rl j rh l", rl=NRL, l=NLO
                        )
                        nc.scalar.dma_start(out=dst, in_=oq[:])

    nc.compile()
    nc.finalize()
    return nc


_NC_CACHE: dict[int, object] = {}


def _get_nc(rows: int):
    if rows not in _NC_CACHE:
        _NC_CACHE[rows] = build_nc(rows)
    return _NC_CACHE[rows]


def run(x: np.ndarray, seed: np.ndarray, trace: bool = False):
    x = np.ascontiguousarray(x, dtype=np.float32)
    seed_t = np.ascontiguousarray(seed.reshape(NHI, NLO).astype(np.float32))
    nc = _get_nc(R_CORE)
    in_maps = [
        {"x": x[i * R_CORE : (i + 1) * R_CORE], "seed": seed_t} for i in range(N_CORES)
    ]
    res = run_bass_kernel_spmd(nc, in_maps, core_ids=list(range(N_CORES)), trace=trace)
    out = np.concatenate([res.results[i]["y"] for i in range(N_CORES)], axis=0)
    return out, res


def kernel(x: np.ndarray, seed: np.ndarray) -> np.ndarray:
    out, _ = run(x, seed)
    return out



# revision 20
# speedup vs baseline: 1.0434x; 1.0434x over previous
"""Randomized Hadamard transform kernel for Trainium2 (8 NeuronCores, SPMD).

Math: out = FWHT(x * seed) / sqrt(4096); the reference butterfly equals the
Sylvester Hadamard matrix, and H_4096 = H_S (x) H_L with c = hi*L + lo
(S = 16, L = 256 here):

    out[r, j*L + m] = (1/64) * sum_{hi,lo} H_S[hi,j] H_L[lo,m] x~[r, hi*L+lo]

Layout trick: matmul computes out[m, n] = sum_k lhsT[k, m] * rhs[k, n].
Pass 1 keeps the *data* as stationary lhsT with the small Hadamard factor as
moving rhs: one MM contracts the data's partition index (rl,hi) and rotates a
128-wide lo-window onto the output partitions.  Pass 2 contracts lo (= L via
two PSUM-accumulated matmuls of 128) with H_L as a 256-wide moving rhs in
fp32r (1 cyc/row at free >= 256) — no explicit transposes anywhere.

Per 128-row tile (rows r = r0 + rh*8 + rl, cols c = hi*256 + lo):
  load   Lt[(rl,hi), (rh,lo)] <- x   in 4 quarter-DMAs (32-row slabs),
         1KB-contiguous chunks (lo runs 256 floats)
  mult   Xt = bf16(Lt * S_rep) per quarter (DVE/GpSimd alternating)
  pass1  slice s=(rh,lo_chunk): psum[lo_sub, (rl,j)] = Xt_s^T @ (I8 (x) H16)
  pass2  per rh: psum2[(rl,j), m] = sum_chunk w_chunk^T @ (H256/64)  (fp32r)
  store  O[(rl,j), (rh,m)] -> out in 2 half-DMAs, 1KB-contiguous chunks
"""

import ml_dtypes
import numpy as np

import concourse.mybir as mybir
from concourse import bacc
import concourse.tile as tile
from concourse.bass_utils import run_bass_kernel_spmd

N_CORES = 8
R_FULL = 8192
C = 4096
R_CORE = R_FULL // N_CORES  # 1024 rows per core
P = 128
NHI, NLO = 16, 256  # c = hi*256 + lo
NRL, NRH = 8, 16  # tile rows r = rh*8 + rl
QF = 1024  # quarter free-size (4 rh values x 256 lo)


def _sylvester(n: int) -> np.ndarray:
    h = np.array([[1.0]], dtype=np.float64)
    while h.shape[0] < n:
        h = np.block([[h, h], [h, -h]])
    return h


def _consts():
    k1 = np.kron(np.eye(NRL), _sylvester(NHI)).astype(ml_dtypes.bfloat16)
    # H256/64 stored as two stacked [128, 256] blocks (contraction chunks)
    k2 = (_sylvester(NLO) / 64.0).astype(np.float32)  # [256, 256]
    return k1, k2


def build_nc(rows: int = R_CORE):
    assert rows % P == 0
    n_tiles = rows // P

    k1_np, k2_np = _consts()

    nc = bacc.Bacc("TRN2", target_bir_lowering=False, debug=False)
    x_in = nc.dram_tensor("x", [rows, C], mybir.dt.float32, kind="ExternalInput")
    s_in = nc.dram_tensor("seed", [NHI, NLO], mybir.dt.float32, kind="ExternalInput")
    y_out = nc.dram_tensor("y", [rows, C], mybir.dt.float32, kind="ExternalOutput")
    k1_dram = nc.inline_tensor(k1_np, "k1")
    k2_dram = nc.inline_tensor(k2_np.reshape(2, P, NLO), "k2")

    f32 = mybir.dt.float32
    f32r = mybir.dt.float32r
    bf16 = mybir.dt.bfloat16

    with tile.TileContext(nc) as tc:
        with (
            tc.tile_pool(name="consts", bufs=1) as cpool,
            tc.tile_pool(name="lt", bufs=6) as lt_pool,
            tc.tile_pool(name="xt", bufs=6) as xt_pool,
            tc.tile_pool(name="w", bufs=2) as w_pool,
            tc.tile_pool(name="o", bufs=6) as o_pool,
            tc.tile_pool(name="ps1", bufs=4, space="PSUM") as ps1_pool,
            tc.tile_pool(name="ps2", bufs=4, space="PSUM") as ps2_pool,
        ):
            k1 = cpool.tile([P, P], bf16)
            k2 = cpool.tile([P, 2 * NLO], f32)  # [:, 0:256]=block0, [:, 256:512]=block1
            srep = cpool.tile([P, QF], f32)
            # constants ride the Scalar HWDGE ring so the first x load
            # starts immediately on the Sync ring
            nc.scalar.dma_start(out=k1[:], in_=k1_dram[:])
            for b in range(2):
                nc.scalar.dma_start(
                    out=k2[:, b * NLO : (b + 1) * NLO], in_=k2_dram[b]
                )
            # engine SBUF access must start at a partition multiple of 32, so
            # DMA the seed tile into both 16-partition halves of the first
            # quadrant, then double 32->64->128 partitions with aligned copies
            nc.scalar.dma_start(out=srep[:NHI, :NLO], in_=s_in[:])
            nc.scalar.dma_start(out=srep[NHI : 2 * NHI, :NLO], in_=s_in[:])
            for d in range(2):
                p0 = 32 << d
                nc.vector.tensor_copy(out=srep[p0 : 2 * p0, :NLO], in_=srep[:p0, :NLO])
            for d in range(2):
                w0 = NLO << d
                nc.vector.tensor_copy(out=srep[:, w0 : 2 * w0], in_=srep[:, :w0])

            for t in range(n_tiles):
                r0 = t * P
                # ---- load + seed multiply (fp32 -> bf16), in 4 quarters
                xtq = []
                for qi in range(4):
                    ltq = lt_pool.tile([P, QF], f32, tag="ltq")
                    src = x_in[r0 + 32 * qi : r0 + 32 * (qi + 1), :].rearrange(
                        "(rh rl) (hi lo) -> rl hi rh lo", rl=NRL, lo=NLO
                    )
                    nc.sync.dma_start(out=ltq[:], in_=src)
                    xq = xt_pool.tile([P, QF], bf16, tag="xtq")
                    # alternate the seed-multiply between DVE and GpSimd
                    eng = nc.vector if qi % 2 == 0 else nc.gpsimd
                    eng.tensor_mul(out=xq[:], in0=ltq[:], in1=srep[:])
                    xtq.append(xq)

                # ---- pass 1: contract (rl,hi) with I8 (x) H16; lo_sub -> partitions
                # free slice s = (rh, lo_chunk); psum_s[lo_sub, (rl,j)]
                w = w_pool.tile([P, C], f32r)
                for g in range(8):
                    ps = ps1_pool.tile([P, 512], f32)
                    for q in range(4):
                        s = 4 * g + q
                        nc.tensor.matmul(
                            ps[:, q * P : (q + 1) * P],
                            lhsT=xtq[s // 8][:, (s % 8) * P : (s % 8 + 1) * P],
                            rhs=k1[:],
                            start=True,
                            stop=True,
                        )
                    if g % 2 == 0:
                        nc.vector.tensor_copy(out=w[:, g * 512 : (g + 1) * 512], in_=ps[:])
                    else:
                        nc.scalar.copy(out=w[:, g * 512 : (g + 1) * 512], in_=ps[:])

                # ---- pass 2: contract lo (256) with H256/64 in fp32r;
                # per rh: psum2[(rl,j), m] accumulated over 2 lo-chunks
                for h in range(2):
                    oh = o_pool.tile([P, 2048], f32, tag="oh", name=f"oh{t}_{h}")
                    for pr in range(4):  # pair of rh values per PSUM bank
                        ps = ps2_pool.tile([P, 512], f32)
                        for sub in range(2):
                            rh = 8 * h + 2 * pr + sub
                            for chunk in range(2):
                                s = 2 * rh + chunk
                                nc.tensor.matmul(
                                    ps[:, sub * NLO : (sub + 1) * NLO],
                                    lhsT=w[:, s * P : (s + 1) * P],
                                    rhs=k2[:, chunk * NLO : (chunk + 1) * NLO].bitcast(
                                        f32r
                                    ),
                                    start=(chunk == 0),
                                    stop=(chunk == 1),
                                )
                        dst_sb = oh[:, pr * 512 : (pr + 1) * 512]
                        if pr % 2 == 0:
                            nc.vector.tensor_copy(out=dst_sb, in_=ps[:])
                        else:
                            nc.scalar.copy(out=dst_sb, in_=ps[:])
                    # ---- store halves as soon as each is drained
                    dst = y_out[r0 + 64 * h : r0 + 64 * (h + 1), :].rearrange(
                        "(rh rl) (j m) -> rl j rh m", rl=NRL, m=NLO
                    )
                    nc.scalar.dma_start(out=dst, in_=oh[:])

    nc.compile()
    nc.finalize()
    return nc


_NC_CACHE: dict[int, object] = {}


def _get_nc(rows: int):
    if rows not in _NC_CACHE:
        _NC_CACHE[rows] = build_nc(rows)
    return _NC_CACHE[rows]


def run(x: np.ndarray, seed: np.ndarray, trace: bool = False):
    x = np.ascontiguousarray(x, dtype=np.float32)
    seed_t = np.ascontiguousarray(seed.reshape(NHI, NLO).astype(np.float32))
    nc = _get_nc(R_CORE)
    in_maps = [
        {"x": x[i * R_CORE : (i + 1) * R_CORE], "seed": seed_t} for i in range(N_CORES)
    ]
    res = run_bass_kernel_spmd(nc, in_maps, core_ids=list(range(N_CORES)), trace=trace)
    out = np.concatenate([res.results[i]["y"] for i in range(N_CORES)], axis=0)
    return out, res


def kernel(x: np.ndarray, seed: np.ndarray) -> np.ndarray:
    out, _ = run(x, seed)
    return out
